# revision 16
# baseline (speedup 1.0000x reference)
import sys
sys.path.insert(0, "/opt/trn_rl_repo")
import numpy as np
from contextlib import ExitStack

import concourse.bass as bass
import concourse.bacc as bacc
import concourse.tile as tile
from concourse import mybir
from concourse.bass_utils import run_bass_kernel_spmd

# Problem constants (hardcoded per spec)
B, C, H, W = 4, 64, 256, 256
K, PAD, DG = 3, 1, 4
Co = DG * 2 * K * K  # 72
N_CORES = 8
ROWS = 128            # output rows per core
XR = 161              # xcore rows: [r0-16, r0+145)
M_OFF = 12            # gather window margin
WIN = 33              # window rows per half
NE = WIN * 256        # ap_gather num_elems (d-blocks)
NB = 8                # blocks per core
R = 16                # rows per block
NPOS = R * 256        # 4096 positions per block
NIDX = 1024           # samples per gather call per 16-partition group
f32, bf16, i16, i32, u32 = (mybir.dt.float32, mybir.dt.bfloat16,
                            mybir.dt.int16, mybir.dt.int32, mybir.dt.uint32)

_CACHE = {}


def _q_of(m, g, c):
    # t-slab partition for tap m, group g, coord c (0=y,1=x): q = c*36 + m*4 + g
    return c * 36 + m * 4 + g


def _orig_ch(m, g, c):
    # original offset-channel in reference layout: g*18 + m*2 + c
    return g * 18 + m * 2 + c


def build_program():
    nc = bacc.Bacc("TRN2", target_bir_lowering=False, debug=False,
                   num_devices=N_CORES)
    xc_ap = nc.dram_tensor("xc", [64, XR * 256], f32, kind="ExternalInput").ap()
    wp_ap = nc.dram_tensor("wp", [64, 9 * 128], bf16, kind="ExternalInput").ap()
    bias_ap = nc.dram_tensor("biasp", [73, 1], f32, kind="ExternalInput").ap()
    cb_ap = nc.dram_tensor("cbase", [72, 1024], f32, kind="ExternalInput").ap()
    b4_ap = nc.dram_tensor("bias4", [72, 4], f32, kind="ExternalInput").ap()
    cl_ap = nc.dram_tensor("cl", [72, 2], f32, kind="ExternalInput").ap()
    msk_ap = nc.dram_tensor("mskd", [1, ROWS * 256], bf16,
                            kind="ExternalInput").ap()
    la_ap = nc.dram_tensor("lA", [128, 9 * 64], bf16, kind="ExternalInput").ap()
    lb_ap = nc.dram_tensor("lB", [64, 64], bf16, kind="ExternalInput").ap()
    lr_ap = nc.dram_tensor("lR", [72, 64], bf16, kind="ExternalInput").ap()
    out_ap = nc.dram_tensor("outc", [64, ROWS * 256], f32, kind="ExternalOutput").ap()

    imgq_d = nc.dram_tensor("imgq_d", [64, 160 * 512], u32).ap()

    with ExitStack() as ctx:
        tc = ctx.enter_context(tile.TileContext(nc))
        cpool = ctx.enter_context(tc.tile_pool(name="consts", bufs=1))
        wp_t = cpool.tile([64, 9 * 128], bf16, tag="wp")
        nc.sync.dma_start(wp_t[:], wp_ap[:])
        bias_t = cpool.tile([73, 1], f32, tag="bias")
        nc.sync.dma_start(bias_t[:], bias_ap[:])
        cb_t = cpool.tile([72, 1024], f32, tag="cb")
        nc.sync.dma_start(cb_t[:], cb_ap[:])
        b4_t = cpool.tile([72, 4], f32, tag="b4")
        nc.sync.dma_start(b4_t[:], b4_ap[:])
        cl_t = cpool.tile([72, 2], f32, tag="cl")
        nc.sync.dma_start(cl_t[:], cl_ap[:])
        la_t = cpool.tile([128, 9 * 64], bf16, tag="la")
        nc.sync.dma_start(la_t[:], la_ap[:])
        lb_t = cpool.tile([64, 64], bf16, tag="lb")
        nc.sync.dma_start(lb_t[:], lb_ap[:])
        lr_t = cpool.tile([72, 64], bf16, tag="lr")
        nc.sync.dma_start(lr_t[:], lr_ap[:])
        cm1_t = cpool.tile([72, 1], f32, tag="cm1")
        nc.vector.memset(cm1_t[:], -1.0)

        # ---- prepass: build imgq_d (bf16 pair-quad image) ----
        with tc.tile_pool(name="prep", bufs=2) as ppool:
            for cch in range(8):
                r0 = 20 * cch
                xp = ppool.tile([64, 21 * 257], f32, tag="xp")
                xpv = xp[:].rearrange("p (r w) -> p r w", w=257)
                nc.sync.dma_start(
                    xpv[:, :, 0:256],
                    xc_ap[:, r0 * 256:(r0 + 21) * 256].rearrange(
                        "p (r w) -> p r w", w=256))
                nc.vector.memset(xpv[:, :, 256:257], 0.0)
                qt = ppool.tile([64, 20 * 512], u32, tag="qt")
                qv = qt[:].bitcast(bf16).rearrange(
                    "p (r w k e) -> p r w k e", w=256, k=2, e=2)
                nc.vector.tensor_copy(qv[:, :, :, 0, 0], xpv[:, 0:20, 0:256])
                nc.scalar.copy(qv[:, :, :, 0, 1], xpv[:, 0:20, 1:257])
                nc.vector.tensor_copy(qv[:, :, :, 1, 0], xpv[:, 1:21, 0:256])
                nc.scalar.copy(qv[:, :, :, 1, 1], xpv[:, 1:21, 1:257])
                nc.sync.dma_start(imgq_d[:, r0 * 512:(r0 + 20) * 512], qt[:])

        # ---- block pools ----
        qwpool = ctx.enter_context(tc.tile_pool(name="qw", bufs=1))
        xwpool = ctx.enter_context(tc.tile_pool(name="xw", bufs=1))
        tpool = ctx.enter_context(tc.tile_pool(name="tt", bufs=2))
        psA = ctx.enter_context(tc.tile_pool(name="psA", bufs=1, space="PSUM"))
        psB = ctx.enter_context(tc.tile_pool(name="psB", bufs=1, space="PSUM"))
        psF = ctx.enter_context(tc.tile_pool(name="psF", bufs=2, space="PSUM"))
        ch_pool = ctx.enter_context(tc.tile_pool(name="chain", bufs=1))
        al_pool = ctx.enter_context(tc.tile_pool(name="aligned", bufs=1))
        ix_pool = ctx.enter_context(tc.tile_pool(name="ixp", bufs=1))
        w4pool = ctx.enter_context(tc.tile_pool(name="w4p", bufs=1))
        gpool = ctx.enter_context(tc.tile_pool(name="gp", bufs=2))
        bpool = ctx.enter_context(tc.tile_pool(name="bp", bufs=2))
        spool = ctx.enter_context(tc.tile_pool(name="sp", bufs=2))
        opool = ctx.enter_context(tc.tile_pool(name="op", bufs=1))
        mpool = ctx.enter_context(tc.tile_pool(name="mp", bufs=1))
        itpool = ctx.enter_context(tc.tile_pool(name="it", bufs=2))
        drpool = ctx.enter_context(tc.tile_pool(name="dr", bufs=2, space="DRAM"))

        for b in range(NB):
            bs = 16 * b

            # imgQ window: xcore rows [bs+4, bs+45), duplicated to both halves
            qw = qwpool.tile([128, WIN * 512], u32, tag="qw")
            nc.sync.dma_start(qw[0:64, :],
                              imgq_d[:, (bs + 4) * 512:(bs + 37) * 512])
            nc.sync.dma_start(qw[64:128, :],
                              imgq_d[:, (bs + 12) * 512:(bs + 45) * 512])

            # t conv + chain, per 1024-pos chunk (4 rows)
            idxcm = ix_pool.tile([36, NPOS], i16, tag="idxcm")
            w4d_b = drpool.tile([36, NPOS * 4], bf16, tag="w4d")
            tmask = mpool.tile([72, NPOS], bf16, tag="tmask")
            xblk_f = mpool.tile([64, NPOS], bf16, tag="xblkf")
            for ck in range(4):
                xw = xwpool.tile([64, 6 * 258], f32, tag="xw")
                xwv = xw[:].rearrange("p (r w) -> p r w", w=258)
                nc.sync.dma_start(
                    xwv[:, :, 1:257],
                    xc_ap[:, (bs + 15 + 4 * ck) * 256:
                          (bs + 21 + 4 * ck) * 256].rearrange(
                        "p (r w) -> p r w", w=256))
                nc.vector.memset(xwv[:, :, 0:1], 0.0)
                nc.vector.memset(xwv[:, :, 257:258], 0.0)
                nc.scalar.copy(
                    xblk_f[:, ck * 1024:(ck + 1) * 1024],
                    xwv[:, 1:5, 1:257])
                xwb = xwpool.tile([64, 6 * 258], bf16, tag="xwb")
                nc.scalar.copy(xwb[:], xw[:])
                xwbv = xwb[:].rearrange("p (r w) -> p r w", w=258)
                ps = (psA if ck % 2 == 0 else psB).tile([73, 1024], f32,
                                                        tag="pst")
                for m in range(9):
                    dy, dx = m // 3 - 1, m % 3 - 1
                    for sub in range(2):
                        # rows of this sub-matmul: 2 rows = 512 cols
                        rr = 1 + dy + 2 * sub
                        rhs = xwbv[:, rr:rr + 2, 1 + dx:1 + dx + 256]
                        nc.tensor.matmul(ps[:, sub * 512:(sub + 1) * 512],
                                         wp_t[:, m * 128:m * 128 + 73],
                                         rhs, start=(m == 0), stop=(m == 8))
                t_c = tpool.tile([73, 1024], f32, tag="tc")
                nc.scalar.activation(t_c[:], ps[:],
                                     mybir.ActivationFunctionType.Identity,
                                     bias=bias_t[:])

                # host-precomputed mask plane -> bcast -> tmask chunk
                mkb = mpool.tile([72, 1024], bf16, tag="mkb")
                nc.scalar.dma_start(
                    mkb[:],
                    msk_ap[0:1, b * NPOS + ck * 1024:
                           b * NPOS + (ck + 1) * 1024].broadcast_to(
                        (72, 1024)))
                nc.vector.tensor_tensor(
                    tmask[:, ck * 1024:(ck + 1) * 1024], t_c[0:72, :],
                    mkb[:], op=mybir.AluOpType.mult)

                # ---- chain ----
                P = ch_pool.tile([72, 1024], f32, tag="P")
                nc.vector.tensor_tensor(P[:], t_c[0:72, :], cb_t[:],
                                        op=mybir.AluOpType.add)
                if ck > 0:
                    nc.vector.tensor_scalar(P[:], P[:], b4_t[:, ck:ck + 1],
                                            None, op0=mybir.AluOpType.add)
                Q = ch_pool.tile([72, 1024], f32, tag="B")
                nc.scalar.activation(Q[:], P[:],
                                     mybir.ActivationFunctionType.Copy,
                                     bias=-0.5, scale=1.0)
                I = ch_pool.tile([72, 1024], i32, tag="I")
                nc.vector.tensor_copy(I[:], Q[:])
                Jf = ch_pool.tile([72, 1024], f32, tag="Jf")
                nc.vector.tensor_copy(Jf[:], I[:])
                nc.vector.tensor_scalar(Jf[:], Jf[:], cl_t[:, 0:1],
                                        cl_t[:, 1:2],
                                        op0=mybir.AluOpType.max,
                                        op1=mybir.AluOpType.min)
                U = ch_pool.tile([72, 1024], f32, tag="B")
                nc.vector.tensor_tensor(U[:], P[:], Jf[:],
                                        op=mybir.AluOpType.subtract)
                # wA = relu(min(1-u, 1+u)) = relu(1-|u|)
                # wB = relu(min(2-u, u))   = relu(1-|u-1|)
                A1 = ch_pool.tile([72, 1024], f32, tag="I")
                nc.scalar.activation(A1[:], U[:],
                                     mybir.ActivationFunctionType.Abs,
                                     bias=0.0, scale=1.0)
                WA = ch_pool.tile([72, 1024], f32, tag="WA")
                nc.scalar.activation(WA[:], A1[:],
                                     mybir.ActivationFunctionType.Relu,
                                     bias=1.0, scale=-1.0)
                A2 = ch_pool.tile([72, 1024], f32, tag="F")
                nc.scalar.activation(A2[:], U[:],
                                     mybir.ActivationFunctionType.Abs,
                                     bias=cm1_t[:], scale=1.0)
                WB = ch_pool.tile([72, 1024], f32, tag="WB")
                nc.scalar.activation(WB[:], A2[:],
                                     mybir.ActivationFunctionType.Relu,
                                     bias=1.0, scale=-1.0)

                # align x-side rows [36:72] down to partitions 0:36
                jx = al_pool.tile([36, 1024], f32, tag="jx")
                nc.scalar.dma_start(jx[:], Jf[36:72, :])
                wxA = al_pool.tile([36, 1024], f32, tag="wxA")
                nc.scalar.dma_start(wxA[:], WA[36:72, :])
                wxB = al_pool.tile([36, 1024], f32, tag="wxB")
                nc.scalar.dma_start(wxB[:], WB[36:72, :])

                # idx = Jy*256 + Jx - 16  (f32 exact), then -> int16
                af = ch_pool.tile([36, 1024], f32, tag="P")
                nc.vector.tensor_scalar(af[:], Jf[0:36, :], 256.0, -16.0,
                                        op0=mybir.AluOpType.mult,
                                        op1=mybir.AluOpType.add)
                nc.vector.tensor_tensor(af[:], af[:], jx[:],
                                        op=mybir.AluOpType.add)
                dst_v = idxcm[:, ck * 1024:(ck + 1) * 1024].rearrange(
                    "p (r s) -> p r s", r=16, s=64)
                src_v = af[:].rearrange("p (s r) -> p r s", s=64, r=16)
                nc.vector.tensor_copy(dst_v, src_v)

                # W4 quad (interleaved bf16): order (yA*xA, yA*xB, yB*xA, yB*xB)
                w4c = w4pool.tile([36, 1024 * 4], bf16, tag="w4c")
                w4v = w4c[:].rearrange("p (n k) -> p n k", k=4)
                nc.vector.tensor_tensor(w4v[:, :, 0], WA[0:36, :], wxA[:],
                                        op=mybir.AluOpType.mult)
                nc.vector.tensor_tensor(w4v[:, :, 1], WA[0:36, :], wxB[:],
                                        op=mybir.AluOpType.mult)
                nc.vector.tensor_tensor(w4v[:, :, 2], WB[0:36, :], wxA[:],
                                        op=mybir.AluOpType.mult)
                nc.vector.tensor_tensor(w4v[:, :, 3], WB[0:36, :], wxB[:],
                                        op=mybir.AluOpType.mult)
                nc.scalar.dma_start(
                    w4d_b[:, ck * 4096:(ck + 1) * 4096], w4c[:])


            # gather + blend + final conv
            for q in range(2):
                psq0 = psF.tile([64, 1024], f32, tag="psf")
                psq1 = psF.tile([64, 1024], f32, tag="psf")
                psq = [psq0, psq1]
                first_mm = [True, True]
                for m in range(9):
                    it = itpool.tile([128, 64], i16, tag="it")
                    for hf in range(2):
                        off = hf * 2048 + q * 1024
                        srcv = idxcm[m * 4:(m + 1) * 4,
                                     off:off + 1024].rearrange(
                                         "g (r s) -> g r s", s=64)
                        nc.sync.dma_start(it[hf * 64:(hf + 1) * 64, :], srcv)
                    w4b = bpool.tile([128, 4096], bf16, tag="w4b")
                    for hf in range(2):
                        off = (hf * 2048 + q * 1024) * 4
                        src3 = w4d_b[m * 4:(m + 1) * 4, off:off + 4096]
                        src3 = src3[:, None, :].broadcast_to((4, 16, 4096))
                        nc.scalar.dma_start(
                            w4b[hf * 64:(hf + 1) * 64, :], src3)
                    gt = gpool.tile([128, NIDX * 2], u32, tag="gt")
                    nc.gpsimd.ap_gather(gt[:], qw[:], it[:], channels=128,
                                        num_elems=NE, d=2, num_idxs=NIDX)
                    gb = gt[:].bitcast(bf16)
                    nc.vector.tensor_tensor(gb, gb, w4b[:],
                                            op=mybir.AluOpType.mult)
                    g2 = gb.rearrange("p (n k) -> p n k", k=2)
                    t1 = gb[:, 0:NIDX * 2]
                    nc.vector.tensor_tensor(t1, g2[:, :, 0], g2[:, :, 1],
                                            op=mybir.AluOpType.add)
                    t2 = t1.rearrange("p (n k) -> p n k", k=2)
                    sm = spool.tile([128, NIDX], bf16, tag="sm")
                    nc.vector.tensor_tensor(sm[:], t2[:, :, 0], t2[:, :, 1],
                                            op=mybir.AluOpType.add)
                    for hf in range(2):
                        for qq in range(2):
                            nc.tensor.matmul(
                                psq[hf][:, qq * 512:(qq + 1) * 512],
                                la_t[hf * 64:(hf + 1) * 64,
                                     m * 64:(m + 1) * 64],
                                sm[hf * 64:(hf + 1) * 64,
                                   qq * 512:(qq + 1) * 512],
                                start=first_mm[hf], stop=False)
                        first_mm[hf] = False
                # B and R2 terms for this q-region, then tanh + out
                for hf in range(2):
                    for qq in range(2):
                        cols = slice(hf * 2048 + q * 1024 + qq * 512,
                                     hf * 2048 + q * 1024 + (qq + 1) * 512)
                        pcols = slice(qq * 512, (qq + 1) * 512)
                        nc.tensor.matmul(psq[hf][:, pcols], lb_t[:],
                                         xblk_f[:, cols],
                                         start=False, stop=False)
                        nc.tensor.matmul(psq[hf][:, pcols], lr_t[:],
                                         tmask[:, cols], start=False,
                                         stop=True)
                    outsb = opool.tile([64, 1024], f32, tag="outsb")
                    nc.scalar.activation(outsb[:], psq[hf][:],
                                         mybir.ActivationFunctionType.Tanh)
                    nc.sync.dma_start(
                        out_ap[:, b * NPOS + hf * 2048 + q * 1024:
                               b * NPOS + hf * 2048 + (q + 1) * 1024],
                        outsb[:])

    nc.compile()
    return nc


def _prep_consts(conv_o_w, conv_o_b, conv_m_w, conv_r_w):
    import ml_dtypes
    # permuted conv weights: lhsT [64, 9*128]: per tap m cols [0..73)
    Wmod = conv_o_w.copy()
    Wmod[:, :, 1, 1] -= conv_o_w.sum((2, 3))
    wp = np.zeros((64, 9 * 128), np.float32)
    biasp = np.zeros((73, 1), np.float32)
    for mt in range(9):          # conv tap
        dy, dx = mt // 3, mt % 3
        for mq in range(9):      # output offset-channel tap slot
            for g in range(DG):
                for c in range(2):
                    q = _q_of(mq, g, c)
                    oc = _orig_ch(mq, g, c)
                    wp[:, mt * 128 + q] = Wmod[oc, :, dy, dx]
    for mq in range(9):
        for g in range(DG):
            for c in range(2):
                biasp[_q_of(mq, g, c), 0] = conv_o_b[_orig_ch(mq, g, c)]
    wp[:, 4 * 128 + 72] = 1.0  # xsum via center tap
    biasp[72, 0] = 0.0

    cbase = np.zeros((72, 1024), np.float32)
    n = np.arange(1024)
    for m in range(9):
        i, j = m // 3, m % 3
        for g in range(DG):
            cbase[_q_of(m, g, 0)] = (n // 256) + M_OFF + (i - 1)
            cbase[_q_of(m, g, 1)] = (n % 256) + 16 + (j - 1)
    bias4 = np.zeros((72, 4), np.float32)
    bias4[0:36] = np.array([0.0, 4.0, 0.0, 4.0])[None, :]
    cl = np.zeros((72, 2), np.float32)
    cl[0:36, 0] = 0.0
    cl[0:36, 1] = WIN - 2.0        # y clamp [0, 31]
    cl[36:72, 0] = 16.0
    cl[36:72, 1] = 270.0           # x clamp
    R1 = conv_r_w[:, 0:64]
    R2 = conv_r_w[:, 64:64 + Co]
    wdw = conv_m_w.reshape(64, 9)
    sig = conv_m_w.sum((1, 2, 3))
    lA = np.zeros((64, 9 * 64), np.float32)
    for m in range(9):
        lA[:, m * 64:(m + 1) * 64] = (R1 * wdw[None, :, m]).T  # [c, o]
    lB = (-R1 * sig[None, :]).T
    lR = np.zeros((72, 64), np.float32)
    for m in range(9):
        for g in range(DG):
            for c in range(2):
                lR[_q_of(m, g, c)] = R2[:, _orig_ch(m, g, c)]
    to_bf = lambda a: a.astype(ml_dtypes.bfloat16)
    lA2 = np.concatenate([lA, lA], axis=0)  # duplicate for partition-64 base
    return (to_bf(wp), biasp, cbase, bias4, cl, to_bf(lA2), to_bf(lB),
            to_bf(lR))


def kernel(x, conv_o_w, conv_o_b, conv_m_w, conv_r_w):
    x = np.asarray(x, np.float32)
    conv_o_w = np.asarray(conv_o_w, np.float32)
    conv_o_b = np.asarray(conv_o_b, np.float32)
    conv_m_w = np.asarray(conv_m_w, np.float32)
    conv_r_w = np.asarray(conv_r_w, np.float32)
    if "nc" not in _CACHE:
        _CACHE["nc"] = build_program()
    nc = _CACHE["nc"]
    wp, biasp, cbase, bias4, cl, lA, lB, lR = _prep_consts(
        conv_o_w, conv_o_b, conv_m_w, conv_r_w)

    import ml_dtypes
    xsum = x.sum(axis=1)  # [B, H, W] exact fp32 mask source
    in_maps = []
    for k in range(N_CORES):
        bb, rh = k // 2, k % 2
        r0 = rh * ROWS
        xcore = np.zeros((64, XR, 256), np.float32)
        lo, hi = r0 - 16, r0 + 145
        slo, shi = max(lo, 0), min(hi, 256)
        xcore[:, slo - lo:shi - lo, :] = x[bb, :, slo:shi, :]
        msk = (xsum[bb, r0:r0 + ROWS, :] > 0).astype(ml_dtypes.bfloat16)
        in_maps.append({
            "xc": np.ascontiguousarray(xcore.reshape(64, XR * 256)),
            "wp": wp, "biasp": biasp, "cbase": cbase, "bias4": bias4,
            "cl": cl, "lA": lA, "lB": lB, "lR": lR,
            "mskd": np.ascontiguousarray(msk.reshape(1, ROWS * 256)),
        })
    res = run_bass_kernel_spmd(nc, in_maps, list(range(N_CORES)))
    out = np.zeros((B, 64, H, W), np.float32)
    for k in range(N_CORES):
        bb, rh = k // 2, k % 2
        out[bb, :, rh * ROWS:(rh + 1) * ROWS, :] = (
            res.results[k]["outc"].reshape(64, ROWS, 256))
    return out



# revision 21
# speedup vs baseline: 1.0180x; 1.0180x over previous
import sys
sys.path.insert(0, "/opt/trn_rl_repo")
import numpy as np
from contextlib import ExitStack

import concourse.bass as bass
import concourse.bacc as bacc
import concourse.tile as tile
from concourse import mybir
from concourse.bass_utils import run_bass_kernel_spmd

# Problem constants (hardcoded per spec)
B, C, H, W = 4, 64, 256, 256
K, PAD, DG = 3, 1, 4
Co = DG * 2 * K * K  # 72
N_CORES = 8
ROWS = 128            # output rows per core
XR = 161              # xcore rows: [r0-16, r0+145)
M_OFF = 12            # gather window margin
WIN = 33              # window rows per half
NE = WIN * 256        # ap_gather num_elems (d-blocks)
NB = 8                # blocks per core
R = 16                # rows per block
NPOS = R * 256        # 4096 positions per block
NIDX = 1024           # samples per gather call per 16-partition group
f32, bf16, i16, i32, u32 = (mybir.dt.float32, mybir.dt.bfloat16,
                            mybir.dt.int16, mybir.dt.int32, mybir.dt.uint32)

_CACHE = {}


def _q_of(m, g, c):
    # t-slab partition for tap m, group g, coord c (0=y,1=x): q = c*36 + m*4 + g
    return c * 36 + m * 4 + g


def _orig_ch(m, g, c):
    # original offset-channel in reference layout: g*18 + m*2 + c
    return g * 18 + m * 2 + c


def build_program():
    nc = bacc.Bacc("TRN2", target_bir_lowering=False, debug=False,
                   num_devices=N_CORES)
    xc_ap = nc.dram_tensor("xc", [64, XR * 256], f32, kind="ExternalInput").ap()
    wp_ap = nc.dram_tensor("wp", [64, 9 * 128], bf16, kind="ExternalInput").ap()
    bias_ap = nc.dram_tensor("biasp", [73, 1], f32, kind="ExternalInput").ap()
    cb_ap = nc.dram_tensor("cbase", [72, 1024], f32, kind="ExternalInput").ap()
    b4_ap = nc.dram_tensor("bias4", [72, 4], f32, kind="ExternalInput").ap()
    cl_ap = nc.dram_tensor("cl", [72, 2], f32, kind="ExternalInput").ap()
    msk_ap = nc.dram_tensor("mskd", [1, ROWS * 256], bf16,
                            kind="ExternalInput").ap()
    la_ap = nc.dram_tensor("lA", [128, 9 * 64], bf16, kind="ExternalInput").ap()
    lb_ap = nc.dram_tensor("lB", [64, 64], bf16, kind="ExternalInput").ap()
    lr_ap = nc.dram_tensor("lR", [72, 64], bf16, kind="ExternalInput").ap()
    out_ap = nc.dram_tensor("outc", [64, ROWS * 256], f32, kind="ExternalOutput").ap()

    imgq_d = nc.dram_tensor("imgq_d", [64, 160 * 512], u32).ap()

    with ExitStack() as ctx:
        tc = ctx.enter_context(tile.TileContext(nc))
        cpool = ctx.enter_context(tc.tile_pool(name="consts", bufs=1))
        wp_t = cpool.tile([64, 9 * 128], bf16, tag="wp")
        nc.sync.dma_start(wp_t[:], wp_ap[:])
        bias_t = cpool.tile([73, 1], f32, tag="bias")
        nc.sync.dma_start(bias_t[:], bias_ap[:])
        cb_t = cpool.tile([72, 1024], f32, tag="cb")
        nc.sync.dma_start(cb_t[:], cb_ap[:])
        b4_t = cpool.tile([72, 4], f32, tag="b4")
        nc.sync.dma_start(b4_t[:], b4_ap[:])
        cl_t = cpool.tile([72, 2], f32, tag="cl")
        nc.sync.dma_start(cl_t[:], cl_ap[:])
        la_t = cpool.tile([128, 9 * 64], bf16, tag="la")
        nc.sync.dma_start(la_t[:], la_ap[:])
        lb_t = cpool.tile([64, 64], bf16, tag="lb")
        nc.sync.dma_start(lb_t[:], lb_ap[:])
        lr_t = cpool.tile([72, 64], bf16, tag="lr")
        nc.sync.dma_start(lr_t[:], lr_ap[:])
        cm1_t = cpool.tile([72, 1], f32, tag="cm1")
        nc.vector.memset(cm1_t[:], -1.0)

        # ---- prepass: build imgq_d (bf16 pair-quad image) ----
        with tc.tile_pool(name="prep", bufs=2) as ppool:
            for cch in range(8):
                r0 = 20 * cch
                xp = ppool.tile([64, 21 * 257], f32, tag="xp")
                xpv = xp[:].rearrange("p (r w) -> p r w", w=257)
                nc.sync.dma_start(
                    xpv[:, :, 0:256],
                    xc_ap[:, r0 * 256:(r0 + 21) * 256].rearrange(
                        "p (r w) -> p r w", w=256))
                nc.vector.memset(xpv[:, :, 256:257], 0.0)
                qt = ppool.tile([64, 20 * 512], u32, tag="qt")
                qv = qt[:].bitcast(bf16).rearrange(
                    "p (r w k e) -> p r w k e", w=256, k=2, e=2)
                nc.vector.tensor_copy(qv[:, :, :, 0, 0], xpv[:, 0:20, 0:256])
                nc.scalar.copy(qv[:, :, :, 0, 1], xpv[:, 0:20, 1:257])
                nc.vector.tensor_copy(qv[:, :, :, 1, 0], xpv[:, 1:21, 0:256])
                nc.scalar.copy(qv[:, :, :, 1, 1], xpv[:, 1:21, 1:257])
                nc.scalar.dma_start(imgq_d[:, r0 * 512:(r0 + 20) * 512],
                                    qt[:])

        # ---- block pools ----
        qwpool = ctx.enter_context(tc.tile_pool(name="qw", bufs=1))
        xwpool = ctx.enter_context(tc.tile_pool(name="xw", bufs=1))
        tpool = ctx.enter_context(tc.tile_pool(name="tt", bufs=2))
        psA = ctx.enter_context(tc.tile_pool(name="psA", bufs=1, space="PSUM"))
        psB = ctx.enter_context(tc.tile_pool(name="psB", bufs=1, space="PSUM"))
        psF = ctx.enter_context(tc.tile_pool(name="psF", bufs=2, space="PSUM"))
        ch_pool = ctx.enter_context(tc.tile_pool(name="chain", bufs=1))
        al_pool = ctx.enter_context(tc.tile_pool(name="aligned", bufs=1))
        ix_pool = ctx.enter_context(tc.tile_pool(name="ixp", bufs=2))
        w4pool = ctx.enter_context(tc.tile_pool(name="w4p", bufs=1))
        gpool = ctx.enter_context(tc.tile_pool(name="gp", bufs=2))
        bpool = ctx.enter_context(tc.tile_pool(name="bp", bufs=2))
        spool = ctx.enter_context(tc.tile_pool(name="sp", bufs=2))
        opool = ctx.enter_context(tc.tile_pool(name="op", bufs=1))
        mpool = ctx.enter_context(tc.tile_pool(name="mp", bufs=1))
        itpool = ctx.enter_context(tc.tile_pool(name="it", bufs=2))
        drpool = ctx.enter_context(tc.tile_pool(name="dr", bufs=2, space="DRAM"))

        for b in range(NB):
            bs = 16 * b

            # imgQ window: xcore rows [bs+4, bs+45), duplicated to both halves
            qw = qwpool.tile([128, WIN * 512], u32, tag="qw")
            nc.sync.dma_start(qw[0:64, :],
                              imgq_d[:, (bs + 4) * 512:(bs + 37) * 512])
            nc.sync.dma_start(qw[64:128, :],
                              imgq_d[:, (bs + 12) * 512:(bs + 45) * 512])

            # t conv + chain, per 1024-pos chunk (4 rows)
            idxcm = ix_pool.tile([36, NPOS], i16, tag="idxcm")
            w4d_b = drpool.tile([36, NPOS * 4], bf16, tag="w4d")
            tmask = mpool.tile([72, NPOS], bf16, tag="tmask")
            xblk_f = mpool.tile([64, NPOS], bf16, tag="xblkf")
            for cki, ck in enumerate((0, 2, 1, 3)):
                xw = xwpool.tile([64, 6 * 258], f32, tag="xw")
                xwv = xw[:].rearrange("p (r w) -> p r w", w=258)
                nc.sync.dma_start(
                    xwv[:, :, 1:257],
                    xc_ap[:, (bs + 15 + 4 * ck) * 256:
                          (bs + 21 + 4 * ck) * 256].rearrange(
                        "p (r w) -> p r w", w=256))
                nc.vector.memset(xwv[:, :, 0:1], 0.0)
                nc.vector.memset(xwv[:, :, 257:258], 0.0)
                nc.scalar.copy(
                    xblk_f[:, ck * 1024:(ck + 1) * 1024],
                    xwv[:, 1:5, 1:257])
                xwb = xwpool.tile([64, 6 * 258], bf16, tag="xwb")
                nc.scalar.copy(xwb[:], xw[:])
                xwbv = xwb[:].rearrange("p (r w) -> p r w", w=258)
                ps = (psA if cki % 2 == 0 else psB).tile([73, 1024], f32,
                                                         tag="pst")
                for m in range(9):
                    dy, dx = m // 3 - 1, m % 3 - 1
                    for sub in range(2):
                        # rows of this sub-matmul: 2 rows = 512 cols
                        rr = 1 + dy + 2 * sub
                        rhs = xwbv[:, rr:rr + 2, 1 + dx:1 + dx + 256]
                        nc.tensor.matmul(ps[:, sub * 512:(sub + 1) * 512],
                                         wp_t[:, m * 128:m * 128 + 73],
                                         rhs, start=(m == 0), stop=(m == 8))
                t_c = tpool.tile([73, 1024], f32, tag="tc")
                nc.scalar.activation(t_c[:], ps[:],
                                     mybir.ActivationFunctionType.Identity,
                                     bias=bias_t[:])

                # host-precomputed mask plane -> bcast -> tmask chunk
                mkb = mpool.tile([72, 1024], bf16, tag="mkb")
                nc.scalar.dma_start(
                    mkb[:],
                    msk_ap[0:1, b * NPOS + ck * 1024:
                           b * NPOS + (ck + 1) * 1024].broadcast_to(
                        (72, 1024)))
                nc.vector.tensor_tensor(
                    tmask[:, ck * 1024:(ck + 1) * 1024], t_c[0:72, :],
                    mkb[:], op=mybir.AluOpType.mult)

                # ---- chain ----
                P = ch_pool.tile([72, 1024], f32, tag="P")
                nc.vector.tensor_tensor(P[:], t_c[0:72, :], cb_t[:],
                                        op=mybir.AluOpType.add)
                if ck > 0:
                    nc.vector.tensor_scalar(P[:], P[:], b4_t[:, ck:ck + 1],
                                            None, op0=mybir.AluOpType.add)
                Q = ch_pool.tile([72, 1024], f32, tag="B")
                nc.scalar.activation(Q[:], P[:],
                                     mybir.ActivationFunctionType.Copy,
                                     bias=-0.5, scale=1.0)
                I = ch_pool.tile([72, 1024], i32, tag="I")
                nc.vector.tensor_copy(I[:], Q[:])
                Jf = ch_pool.tile([72, 1024], f32, tag="Jf")
                nc.vector.tensor_copy(Jf[:], I[:])
                nc.vector.tensor_scalar(Jf[:], Jf[:], cl_t[:, 0:1],
                                        cl_t[:, 1:2],
                                        op0=mybir.AluOpType.max,
                                        op1=mybir.AluOpType.min)
                U = ch_pool.tile([72, 1024], f32, tag="B")
                nc.vector.tensor_tensor(U[:], P[:], Jf[:],
                                        op=mybir.AluOpType.subtract)

                # align x-side rows [36:72] down to partitions 0:36
                jx = al_pool.tile([36, 1024], f32, tag="jx")
                nc.scalar.dma_start(jx[:], Jf[36:72, :])

                # idx = Jy*256 + Jx - 16  (f32 exact), then -> int16
                af = ch_pool.tile([36, 1024], f32, tag="P")
                nc.vector.tensor_scalar(af[:], Jf[0:36, :], 256.0, -16.0,
                                        op0=mybir.AluOpType.mult,
                                        op1=mybir.AluOpType.add)
                nc.vector.tensor_tensor(af[:], af[:], jx[:],
                                        op=mybir.AluOpType.add)
                dst_v = idxcm[:, ck * 1024:(ck + 1) * 1024].rearrange(
                    "p (r s) -> p r s", r=16, s=64)
                src_v = af[:].rearrange("p (s r) -> p r s", s=64, r=16)
                nc.vector.tensor_copy(dst_v, src_v)

                # wA = relu(min(1-u, 1+u)) = relu(1-|u|)
                # wB = relu(min(2-u, u))   = relu(1-|u-1|)
                A1 = ch_pool.tile([72, 1024], f32, tag="I")
                nc.scalar.activation(A1[:], U[:],
                                     mybir.ActivationFunctionType.Abs,
                                     bias=0.0, scale=1.0)
                WA = ch_pool.tile([72, 1024], f32, tag="Jf")
                nc.scalar.activation(WA[:], A1[:],
                                     mybir.ActivationFunctionType.Relu,
                                     bias=1.0, scale=-1.0)
                A2 = ch_pool.tile([72, 1024], f32, tag="F")
                nc.scalar.activation(A2[:], U[:],
                                     mybir.ActivationFunctionType.Abs,
                                     bias=cm1_t[:], scale=1.0)
                WB = ch_pool.tile([72, 1024], f32, tag="B")
                nc.scalar.activation(WB[:], A2[:],
                                     mybir.ActivationFunctionType.Relu,
                                     bias=1.0, scale=-1.0)

                wxA = al_pool.tile([36, 1024], f32, tag="wxA")
                nc.scalar.dma_start(wxA[:], WA[36:72, :])
                wxB = al_pool.tile([36, 1024], f32, tag="wxB")
                nc.scalar.dma_start(wxB[:], WB[36:72, :])

                # W4 quad (interleaved bf16): order (yA*xA, yA*xB, yB*xA, yB*xB)
                w4c = w4pool.tile([36, 1024 * 4], bf16, tag="w4c")
                w4v = w4c[:].rearrange("p (n k) -> p n k", k=4)
                nc.vector.tensor_tensor(w4v[:, :, 0], WA[0:36, :], wxA[:],
                                        op=mybir.AluOpType.mult)
                nc.vector.tensor_tensor(w4v[:, :, 1], WA[0:36, :], wxB[:],
                                        op=mybir.AluOpType.mult)
                nc.vector.tensor_tensor(w4v[:, :, 2], WB[0:36, :], wxA[:],
                                        op=mybir.AluOpType.mult)
                nc.vector.tensor_tensor(w4v[:, :, 3], WB[0:36, :], wxB[:],
                                        op=mybir.AluOpType.mult)
                nc.scalar.dma_start(
                    w4d_b[:, ck * 4096:(ck + 1) * 4096], w4c[:])


            # gather + blend + final conv
            for q in range(2):
                psq0 = psF.tile([64, 1024], f32, tag="psf")
                psq1 = psF.tile([64, 1024], f32, tag="psf")
                psq = [psq0, psq1]
                first_mm = [True, True]
                for m in range(9):
                    it = itpool.tile([128, 64], i16, tag="it")
                    for hf in range(2):
                        off = hf * 2048 + q * 1024
                        srcv = idxcm[m * 4:(m + 1) * 4,
                                     off:off + 1024].rearrange(
                                         "g (r s) -> g r s", s=64)
                        nc.sync.dma_start(it[hf * 64:(hf + 1) * 64, :], srcv)
                    w4b = bpool.tile([128, 4096], bf16, tag="w4b")
                    for hf in range(2):
                        off = (hf * 2048 + q * 1024) * 4
                        src3 = w4d_b[m * 4:(m + 1) * 4, off:off + 4096]
                        src3 = src3[:, None, :].broadcast_to((4, 16, 4096))
                        nc.scalar.dma_start(
                            w4b[hf * 64:(hf + 1) * 64, :], src3)
                    gt = gpool.tile([128, NIDX * 2], u32, tag="gt")
                    nc.gpsimd.ap_gather(gt[:], qw[:], it[:], channels=128,
                                        num_elems=NE, d=2, num_idxs=NIDX)
                    gb = gt[:].bitcast(bf16)
                    nc.vector.tensor_tensor(gb, gb, w4b[:],
                                            op=mybir.AluOpType.mult)
                    g2 = gb.rearrange("p (n k) -> p n k", k=2)
                    t1 = gb[:, 0:NIDX * 2]
                    nc.vector.tensor_tensor(t1, g2[:, :, 0], g2[:, :, 1],
                                            op=mybir.AluOpType.add)
                    t2 = t1.rearrange("p (n k) -> p n k", k=2)
                    sm = spool.tile([128, NIDX], bf16, tag="sm")
                    nc.vector.tensor_tensor(sm[:], t2[:, :, 0], t2[:, :, 1],
                                            op=mybir.AluOpType.add)
                    for hf in range(2):
                        for qq in range(2):
                            nc.tensor.matmul(
                                psq[hf][:, qq * 512:(qq + 1) * 512],
                                la_t[hf * 64:(hf + 1) * 64,
                                     m * 64:(m + 1) * 64],
                                sm[hf * 64:(hf + 1) * 64,
                                   qq * 512:(qq + 1) * 512],
                                start=first_mm[hf], stop=False)
                        first_mm[hf] = False
                # B and R2 terms for this q-region, then tanh + out
                for hf in range(2):
                    for qq in range(2):
                        cols = slice(hf * 2048 + q * 1024 + qq * 512,
                                     hf * 2048 + q * 1024 + (qq + 1) * 512)
                        pcols = slice(qq * 512, (qq + 1) * 512)
                        nc.tensor.matmul(psq[hf][:, pcols], lb_t[:],
                                         xblk_f[:, cols],
                                         start=False, stop=False)
                        nc.tensor.matmul(psq[hf][:, pcols], lr_t[:],
                                         tmask[:, cols], start=False,
                                         stop=True)
                    outsb = opool.tile([64, 1024], f32, tag="outsb")
                    nc.scalar.activation(outsb[:], psq[hf][:],
                                         mybir.ActivationFunctionType.Tanh)
                    nc.sync.dma_start(
                        out_ap[:, b * NPOS + hf * 2048 + q * 1024:
                               b * NPOS + hf * 2048 + (q + 1) * 1024],
                        outsb[:])

    nc.compile()
    return nc


def _prep_consts(conv_o_w, conv_o_b, conv_m_w, conv_r_w):
    import ml_dtypes
    # permuted conv weights: lhsT [64, 9*128]: per tap m cols [0..73)
    Wmod = conv_o_w.copy()
    Wmod[:, :, 1, 1] -= conv_o_w.sum((2, 3))
    wp = np.zeros((64, 9 * 128), np.float32)
    biasp = np.zeros((73, 1), np.float32)
    for mt in range(9):          # conv tap
        dy, dx = mt // 3, mt % 3
        for mq in range(9):      # output offset-channel tap slot
            for g in range(DG):
                for c in range(2):
                    q = _q_of(mq, g, c)
                    oc = _orig_ch(mq, g, c)
                    wp[:, mt * 128 + q] = Wmod[oc, :, dy, dx]
    for mq in range(9):
        for g in range(DG):
            for c in range(2):
                biasp[_q_of(mq, g, c), 0] = conv_o_b[_orig_ch(mq, g, c)]
    wp[:, 4 * 128 + 72] = 1.0  # xsum via center tap
    biasp[72, 0] = 0.0

    cbase = np.zeros((72, 1024), np.float32)
    n = np.arange(1024)
    for m in range(9):
        i, j = m // 3, m % 3
        for g in range(DG):
            cbase[_q_of(m, g, 0)] = (n // 256) + M_OFF + (i - 1)
            cbase[_q_of(m, g, 1)] = (n % 256) + 16 + (j - 1)
    bias4 = np.zeros((72, 4), np.float32)
    bias4[0:36] = np.array([0.0, 4.0, 0.0, 4.0])[None, :]
    cl = np.zeros((72, 2), np.float32)
    cl[0:36, 0] = 0.0
    cl[0:36, 1] = WIN - 2.0        # y clamp [0, 31]
    cl[36:72, 0] = 16.0
    cl[36:72, 1] = 270.0           # x clamp
    R1 = conv_r_w[:, 0:64]
    R2 = conv_r_w[:, 64:64 + Co]
    wdw = conv_m_w.reshape(64, 9)
    sig = conv_m_w.sum((1, 2, 3))
    lA = np.zeros((64, 9 * 64), np.float32)
    for m in range(9):
        lA[:, m * 64:(m + 1) * 64] = (R1 * wdw[None, :, m]).T  # [c, o]
    lB = (-R1 * sig[None, :]).T
    lR = np.zeros((72, 64), np.float32)
    for m in range(9):
        for g in range(DG):
            for c in range(2):
                lR[_q_of(m, g, c)] = R2[:, _orig_ch(m, g, c)]
    to_bf = lambda a: a.astype(ml_dtypes.bfloat16)
    lA2 = np.concatenate([lA, lA], axis=0)  # duplicate for partition-64 base
    return (to_bf(wp), biasp, cbase, bias4, cl, to_bf(lA2), to_bf(lB),
            to_bf(lR))


def kernel(x, conv_o_w, conv_o_b, conv_m_w, conv_r_w):
    x = np.asarray(x, np.float32)
    conv_o_w = np.asarray(conv_o_w, np.float32)
    conv_o_b = np.asarray(conv_o_b, np.float32)
    conv_m_w = np.asarray(conv_m_w, np.float32)
    conv_r_w = np.asarray(conv_r_w, np.float32)
    if "nc" not in _CACHE:
        _CACHE["nc"] = build_program()
    nc = _CACHE["nc"]
    wp, biasp, cbase, bias4, cl, lA, lB, lR = _prep_consts(
        conv_o_w, conv_o_b, conv_m_w, conv_r_w)

    import ml_dtypes
    xsum = x.sum(axis=1)  # [B, H, W] exact fp32 mask source
    in_maps = []
    for k in range(N_CORES):
        bb, rh = k // 2, k % 2
        r0 = rh * ROWS
        xcore = np.zeros((64, XR, 256), np.float32)
        lo, hi = r0 - 16, r0 + 145
        slo, shi = max(lo, 0), min(hi, 256)
        xcore[:, slo - lo:shi - lo, :] = x[bb, :, slo:shi, :]
        msk = (xsum[bb, r0:r0 + ROWS, :] > 0).astype(ml_dtypes.bfloat16)
        in_maps.append({
            "xc": np.ascontiguousarray(xcore.reshape(64, XR * 256)),
            "wp": wp, "biasp": biasp, "cbase": cbase, "bias4": bias4,
            "cl": cl, "lA": lA, "lB": lB, "lR": lR,
            "mskd": np.ascontiguousarray(msk.reshape(1, ROWS * 256)),
        })
    res = run_bass_kernel_spmd(nc, in_maps, list(range(N_CORES)))
    out = np.zeros((B, 64, H, W), np.float32)
    for k in range(N_CORES):
        bb, rh = k // 2, k % 2
        out[bb, :, rh * ROWS:(rh + 1) * ROWS, :] = (
            res.results[k]["outc"].reshape(64, ROWS, 256))
    return out



# revision 24
# speedup vs baseline: 1.0401x; 1.0217x over previous
import sys
sys.path.insert(0, "/opt/trn_rl_repo")
import numpy as np
from contextlib import ExitStack

import concourse.bass as bass
import concourse.bacc as bacc
import concourse.tile as tile
from concourse import mybir
from concourse.bass_utils import run_bass_kernel_spmd

# Problem constants (hardcoded per spec)
B, C, H, W = 4, 64, 256, 256
K, PAD, DG = 3, 1, 4
Co = DG * 2 * K * K  # 72
N_CORES = 8
ROWS = 128            # output rows per core
XR = 161              # xcore rows: [r0-16, r0+145)
M_OFF = 12            # gather window margin
WIN = 33              # window rows per half
NE = WIN * 256        # ap_gather num_elems (d-blocks)
NB = 8                # blocks per core
R = 16                # rows per block
NPOS = R * 256        # 4096 positions per block
NIDX = 1024           # samples per gather call per 16-partition group
f32, bf16, i16, i32, u32 = (mybir.dt.float32, mybir.dt.bfloat16,
                            mybir.dt.int16, mybir.dt.int32, mybir.dt.uint32)

_CACHE = {}


def _q_of(m, g, c):
    # t-slab partition for tap m, group g, coord c (0=y,1=x): q = c*36 + m*4 + g
    return c * 36 + m * 4 + g


def _orig_ch(m, g, c):
    # original offset-channel in reference layout: g*18 + m*2 + c
    return g * 18 + m * 2 + c


def build_program():
    nc = bacc.Bacc("TRN2", target_bir_lowering=False, debug=False,
                   num_devices=N_CORES)
    xc_ap = nc.dram_tensor("xc", [64, XR * 256], f32, kind="ExternalInput").ap()
    wp_ap = nc.dram_tensor("wp", [64, 9 * 128], bf16, kind="ExternalInput").ap()
    bias_ap = nc.dram_tensor("biasp", [73, 1], f32, kind="ExternalInput").ap()
    cb_ap = nc.dram_tensor("cbase", [72, 1024], f32, kind="ExternalInput").ap()
    b4_ap = nc.dram_tensor("bias4", [72, 4], f32, kind="ExternalInput").ap()
    cl_ap = nc.dram_tensor("cl", [72, 2], f32, kind="ExternalInput").ap()
    msk_ap = nc.dram_tensor("mskd", [1, ROWS * 256], bf16,
                            kind="ExternalInput").ap()
    la_ap = nc.dram_tensor("lA", [128, 9 * 64], bf16, kind="ExternalInput").ap()
    lb_ap = nc.dram_tensor("lB", [64, 64], bf16, kind="ExternalInput").ap()
    lr_ap = nc.dram_tensor("lR", [72, 64], bf16, kind="ExternalInput").ap()
    out_ap = nc.dram_tensor("outc", [64, ROWS * 256], f32, kind="ExternalOutput").ap()

    imgq_d = nc.dram_tensor("imgq_d", [64, 160 * 512], u32).ap()

    with ExitStack() as ctx:
        tc = ctx.enter_context(tile.TileContext(nc))
        cpool = ctx.enter_context(tc.tile_pool(name="consts", bufs=1))
        wp_t = cpool.tile([64, 9 * 128], bf16, tag="wp")
        nc.sync.dma_start(wp_t[:], wp_ap[:])
        bias_t = cpool.tile([73, 1], f32, tag="bias")
        nc.sync.dma_start(bias_t[:], bias_ap[:])
        cb_t = cpool.tile([72, 1024], f32, tag="cb")
        nc.sync.dma_start(cb_t[:], cb_ap[:])
        b4_t = cpool.tile([72, 4], f32, tag="b4")
        nc.sync.dma_start(b4_t[:], b4_ap[:])
        cl_t = cpool.tile([72, 2], f32, tag="cl")
        nc.sync.dma_start(cl_t[:], cl_ap[:])
        la_t = cpool.tile([128, 9 * 64], bf16, tag="la")
        nc.sync.dma_start(la_t[:], la_ap[:])
        lb_t = cpool.tile([64, 64], bf16, tag="lb")
        nc.sync.dma_start(lb_t[:], lb_ap[:])
        lr_t = cpool.tile([72, 64], bf16, tag="lr")
        nc.sync.dma_start(lr_t[:], lr_ap[:])
        cm1_t = cpool.tile([72, 1], f32, tag="cm1")
        nc.vector.memset(cm1_t[:], -1.0)

        # ---- prepass: build imgq_d (bf16 pair-quad image) ----
        with tc.tile_pool(name="prep", bufs=2) as ppool:
            for cch in range(8):
                r0 = 20 * cch
                xp = ppool.tile([64, 21 * 257], f32, tag="xp")
                xpv = xp[:].rearrange("p (r w) -> p r w", w=257)
                nc.sync.dma_start(
                    xpv[:, :, 0:256],
                    xc_ap[:, r0 * 256:(r0 + 21) * 256].rearrange(
                        "p (r w) -> p r w", w=256))
                nc.vector.memset(xpv[:, :, 256:257], 0.0)
                qt = ppool.tile([64, 20 * 512], u32, tag="qt")
                qv = qt[:].bitcast(bf16).rearrange(
                    "p (r w k e) -> p r w k e", w=256, k=2, e=2)
                nc.vector.tensor_copy(qv[:, :, :, 0, 0], xpv[:, 0:20, 0:256])
                nc.scalar.copy(qv[:, :, :, 0, 1], xpv[:, 0:20, 1:257])
                nc.vector.tensor_copy(qv[:, :, :, 1, 0], xpv[:, 1:21, 0:256])
                nc.scalar.copy(qv[:, :, :, 1, 1], xpv[:, 1:21, 1:257])
                nc.scalar.dma_start(imgq_d[:, r0 * 512:(r0 + 20) * 512],
                                    qt[:])

        # ---- block pools ----
        qwpool = ctx.enter_context(tc.tile_pool(name="qw", bufs=1))
        xwpool = ctx.enter_context(tc.tile_pool(name="xw", bufs=1))
        tpool = ctx.enter_context(tc.tile_pool(name="tt", bufs=2))
        psA = ctx.enter_context(tc.tile_pool(name="psA", bufs=1, space="PSUM"))
        psB = ctx.enter_context(tc.tile_pool(name="psB", bufs=1, space="PSUM"))
        psF = ctx.enter_context(tc.tile_pool(name="psF", bufs=2, space="PSUM"))
        ch_pool = ctx.enter_context(tc.tile_pool(name="chain", bufs=1))
        al_pool = ctx.enter_context(tc.tile_pool(name="aligned", bufs=1))
        ix_pool = ctx.enter_context(tc.tile_pool(name="ixp", bufs=2))
        w4pool = ctx.enter_context(tc.tile_pool(name="w4p", bufs=1))
        gpool = ctx.enter_context(tc.tile_pool(name="gp", bufs=2))
        bpool = ctx.enter_context(tc.tile_pool(name="bp", bufs=2))
        spool = ctx.enter_context(tc.tile_pool(name="sp", bufs=2))
        opool = ctx.enter_context(tc.tile_pool(name="op", bufs=1))
        mpool = ctx.enter_context(tc.tile_pool(name="mp", bufs=1))
        itpool = ctx.enter_context(tc.tile_pool(name="it", bufs=2))
        drpool = ctx.enter_context(tc.tile_pool(name="dr", bufs=2, space="DRAM"))

        for b in range(NB):
            bs = 16 * b

            # t conv + chain, per 1024-pos chunk (4 rows)
            idxcm = ix_pool.tile([36, NPOS], i16, tag="idxcm")
            w4d_b = drpool.tile([36, NPOS * 4], bf16, tag="w4d")
            tmask = mpool.tile([72, NPOS], bf16, tag="tmask")
            xblk_f = mpool.tile([64, NPOS], bf16, tag="xblkf")
            for cki, ck in enumerate((0, 2, 1, 3)):
                xw = xwpool.tile([64, 6 * 258], f32, tag="xw")
                xwv = xw[:].rearrange("p (r w) -> p r w", w=258)
                nc.sync.dma_start(
                    xwv[:, :, 1:257],
                    xc_ap[:, (bs + 15 + 4 * ck) * 256:
                          (bs + 21 + 4 * ck) * 256].rearrange(
                        "p (r w) -> p r w", w=256))
                nc.vector.memset(xwv[:, :, 0:1], 0.0)
                nc.vector.memset(xwv[:, :, 257:258], 0.0)
                nc.scalar.copy(
                    xblk_f[:, ck * 1024:(ck + 1) * 1024],
                    xwv[:, 1:5, 1:257])
                xwb = xwpool.tile([64, 6 * 258], bf16, tag="xwb")
                nc.scalar.copy(xwb[:], xw[:])
                xwbv = xwb[:].rearrange("p (r w) -> p r w", w=258)
                ps = (psA if cki % 2 == 0 else psB).tile([73, 1024], f32,
                                                         tag="pst")
                for m in range(9):
                    dy, dx = m // 3 - 1, m % 3 - 1
                    for sub in range(2):
                        # rows of this sub-matmul: 2 rows = 512 cols
                        rr = 1 + dy + 2 * sub
                        rhs = xwbv[:, rr:rr + 2, 1 + dx:1 + dx + 256]
                        nc.tensor.matmul(ps[:, sub * 512:(sub + 1) * 512],
                                         wp_t[:, m * 128:m * 128 + 73],
                                         rhs, start=(m == 0), stop=(m == 8))
                t_c = tpool.tile([73, 1024], f32, tag="tc")
                nc.scalar.activation(t_c[:], ps[:],
                                     mybir.ActivationFunctionType.Identity,
                                     bias=bias_t[:])

                # host-precomputed mask plane -> bcast -> tmask chunk
                mkb = mpool.tile([72, 1024], bf16, tag="mkb")
                nc.scalar.dma_start(
                    mkb[:],
                    msk_ap[0:1, b * NPOS + ck * 1024:
                           b * NPOS + (ck + 1) * 1024].broadcast_to(
                        (72, 1024)))
                nc.vector.tensor_tensor(
                    tmask[:, ck * 1024:(ck + 1) * 1024], t_c[0:72, :],
                    mkb[:], op=mybir.AluOpType.mult)

                # ---- chain ----
                P = ch_pool.tile([72, 1024], f32, tag="P")
                nc.vector.tensor_tensor(P[:], t_c[0:72, :], cb_t[:],
                                        op=mybir.AluOpType.add)
                if ck > 0:
                    nc.vector.tensor_scalar(P[:], P[:], b4_t[:, ck:ck + 1],
                                            None, op0=mybir.AluOpType.add)
                Q = ch_pool.tile([72, 1024], f32, tag="B")
                nc.scalar.activation(Q[:], P[:],
                                     mybir.ActivationFunctionType.Copy,
                                     bias=-0.5, scale=1.0)
                I = ch_pool.tile([72, 1024], i32, tag="I")
                nc.vector.tensor_copy(I[:], Q[:])
                Jf = ch_pool.tile([72, 1024], f32, tag="Jf")
                nc.vector.tensor_copy(Jf[:], I[:])
                nc.vector.tensor_scalar(Jf[:], Jf[:], cl_t[:, 0:1],
                                        cl_t[:, 1:2],
                                        op0=mybir.AluOpType.max,
                                        op1=mybir.AluOpType.min)
                U = ch_pool.tile([72, 1024], f32, tag="B")
                nc.vector.tensor_tensor(U[:], P[:], Jf[:],
                                        op=mybir.AluOpType.subtract)

                # align x-side rows [36:72] down to partitions 0:36
                jx = al_pool.tile([36, 1024], f32, tag="jx")
                nc.scalar.dma_start(jx[:], Jf[36:72, :])

                # idx = Jy*256 + Jx - 16  (f32 exact), then -> int16
                af = ch_pool.tile([36, 1024], f32, tag="P")
                nc.vector.tensor_scalar(af[:], Jf[0:36, :], 256.0, -16.0,
                                        op0=mybir.AluOpType.mult,
                                        op1=mybir.AluOpType.add)
                nc.vector.tensor_tensor(af[:], af[:], jx[:],
                                        op=mybir.AluOpType.add)
                dst_v = idxcm[:, ck * 1024:(ck + 1) * 1024].rearrange(
                    "p (r s) -> p r s", r=16, s=64)
                src_v = af[:].rearrange("p (s r) -> p r s", s=64, r=16)
                nc.vector.tensor_copy(dst_v, src_v)

                # wA = relu(min(1-u, 1+u)) = relu(1-|u|)
                # wB = relu(min(2-u, u))   = relu(1-|u-1|)
                A1 = ch_pool.tile([72, 1024], f32, tag="I")
                nc.scalar.activation(A1[:], U[:],
                                     mybir.ActivationFunctionType.Abs,
                                     bias=0.0, scale=1.0)
                WA = ch_pool.tile([72, 1024], f32, tag="Jf")
                nc.scalar.activation(WA[:], A1[:],
                                     mybir.ActivationFunctionType.Relu,
                                     bias=1.0, scale=-1.0)
                A2 = ch_pool.tile([72, 1024], f32, tag="F")
                nc.scalar.activation(A2[:], U[:],
                                     mybir.ActivationFunctionType.Abs,
                                     bias=cm1_t[:], scale=1.0)
                WB = ch_pool.tile([72, 1024], f32, tag="B")
                nc.scalar.activation(WB[:], A2[:],
                                     mybir.ActivationFunctionType.Relu,
                                     bias=1.0, scale=-1.0)

                wxA = al_pool.tile([36, 1024], f32, tag="wxA")
                nc.scalar.dma_start(wxA[:], WA[36:72, :])
                wxB = al_pool.tile([36, 1024], f32, tag="wxB")
                nc.scalar.dma_start(wxB[:], WB[36:72, :])

                # W4 quad (interleaved bf16): order (yA*xA, yA*xB, yB*xA, yB*xB)
                w4c = w4pool.tile([36, 1024 * 4], bf16, tag="w4c")
                w4v = w4c[:].rearrange("p (n k) -> p n k", k=4)
                nc.vector.tensor_tensor(w4v[:, :, 0], WA[0:36, :], wxA[:],
                                        op=mybir.AluOpType.mult)
                nc.vector.tensor_tensor(w4v[:, :, 1], WA[0:36, :], wxB[:],
                                        op=mybir.AluOpType.mult)
                nc.vector.tensor_tensor(w4v[:, :, 2], WB[0:36, :], wxA[:],
                                        op=mybir.AluOpType.mult)
                nc.vector.tensor_tensor(w4v[:, :, 3], WB[0:36, :], wxB[:],
                                        op=mybir.AluOpType.mult)
                nc.scalar.dma_start(
                    w4d_b[:, ck * 4096:(ck + 1) * 4096], w4c[:])


            # imgQ window: xcore rows [bs+4, bs+45), duplicated to both halves
            # (emitted after the conv chunks so the sync-queue FIFO doesn't
            # head-of-line-block the next block's xw loads behind it)
            qw = qwpool.tile([128, WIN * 512], u32, tag="qw")
            nc.sync.dma_start(qw[0:64, :],
                              imgq_d[:, (bs + 4) * 512:(bs + 37) * 512])
            nc.sync.dma_start(qw[64:128, :],
                              imgq_d[:, (bs + 12) * 512:(bs + 45) * 512])

            # gather + blend + final conv
            for q in range(2):
                psq0 = psF.tile([64, 1024], f32, tag="psf")
                psq1 = psF.tile([64, 1024], f32, tag="psf")
                psq = [psq0, psq1]
                first_mm = [True, True]
                for m in range(9):
                    it = itpool.tile([128, 64], i16, tag="it")
                    for hf in range(2):
                        off = hf * 2048 + q * 1024
                        srcv = idxcm[m * 4:(m + 1) * 4,
                                     off:off + 1024].rearrange(
                                         "g (r s) -> g r s", s=64)
                        nc.sync.dma_start(it[hf * 64:(hf + 1) * 64, :], srcv)
                    w4b = bpool.tile([128, 4096], bf16, tag="w4b")
                    for hf in range(2):
                        off = (hf * 2048 + q * 1024) * 4
                        src3 = w4d_b[m * 4:(m + 1) * 4, off:off + 4096]
                        src3 = src3[:, None, :].broadcast_to((4, 16, 4096))
                        nc.scalar.dma_start(
                            w4b[hf * 64:(hf + 1) * 64, :], src3)
                    gt = gpool.tile([128, NIDX * 2], u32, tag="gt")
                    nc.gpsimd.ap_gather(gt[:], qw[:], it[:], channels=128,
                                        num_elems=NE, d=2, num_idxs=NIDX)
                    gb = gt[:].bitcast(bf16)
                    nc.vector.tensor_tensor(gb, gb, w4b[:],
                                            op=mybir.AluOpType.mult)
                    g2 = gb.rearrange("p (n k) -> p n k", k=2)
                    t1 = gb[:, 0:NIDX * 2]
                    nc.vector.tensor_tensor(t1, g2[:, :, 0], g2[:, :, 1],
                                            op=mybir.AluOpType.add)
                    t2 = t1.rearrange("p (n k) -> p n k", k=2)
                    sm = spool.tile([128, NIDX], bf16, tag="sm")
                    nc.vector.tensor_tensor(sm[:], t2[:, :, 0], t2[:, :, 1],
                                            op=mybir.AluOpType.add)
                    for hf in range(2):
                        for qq in range(2):
                            nc.tensor.matmul(
                                psq[hf][:, qq * 512:(qq + 1) * 512],
                                la_t[hf * 64:(hf + 1) * 64,
                                     m * 64:(m + 1) * 64],
                                sm[hf * 64:(hf + 1) * 64,
                                   qq * 512:(qq + 1) * 512],
                                start=first_mm[hf], stop=False)
                        first_mm[hf] = False
                # B and R2 terms for this q-region, then tanh + out
                for hf in range(2):
                    for qq in range(2):
                        cols = slice(hf * 2048 + q * 1024 + qq * 512,
                                     hf * 2048 + q * 1024 + (qq + 1) * 512)
                        pcols = slice(qq * 512, (qq + 1) * 512)
                        nc.tensor.matmul(psq[hf][:, pcols], lb_t[:],
                                         xblk_f[:, cols],
                                         start=False, stop=False)
                        nc.tensor.matmul(psq[hf][:, pcols], lr_t[:],
                                         tmask[:, cols], start=False,
                                         stop=True)
                    outsb = opool.tile([64, 1024], f32, tag="outsb")
                    nc.scalar.activation(outsb[:], psq[hf][:],
                                         mybir.ActivationFunctionType.Tanh)
                    nc.scalar.dma_start(
                        out_ap[:, b * NPOS + hf * 2048 + q * 1024:
                               b * NPOS + hf * 2048 + (q + 1) * 1024],
                        outsb[:])

    nc.compile()
    return nc


def _prep_consts(conv_o_w, conv_o_b, conv_m_w, conv_r_w):
    import ml_dtypes
    # permuted conv weights: lhsT [64, 9*128]: per tap m cols [0..73)
    Wmod = conv_o_w.copy()
    Wmod[:, :, 1, 1] -= conv_o_w.sum((2, 3))
    wp = np.zeros((64, 9 * 128), np.float32)
    biasp = np.zeros((73, 1), np.float32)
    for mt in range(9):          # conv tap
        dy, dx = mt // 3, mt % 3
        for mq in range(9):      # output offset-channel tap slot
            for g in range(DG):
                for c in range(2):
                    q = _q_of(mq, g, c)
                    oc = _orig_ch(mq, g, c)
                    wp[:, mt * 128 + q] = Wmod[oc, :, dy, dx]
    for mq in range(9):
        for g in range(DG):
            for c in range(2):
                biasp[_q_of(mq, g, c), 0] = conv_o_b[_orig_ch(mq, g, c)]
    wp[:, 4 * 128 + 72] = 1.0  # xsum via center tap
    biasp[72, 0] = 0.0

    cbase = np.zeros((72, 1024), np.float32)
    n = np.arange(1024)
    for m in range(9):
        i, j = m // 3, m % 3
        for g in range(DG):
            cbase[_q_of(m, g, 0)] = (n // 256) + M_OFF + (i - 1)
            cbase[_q_of(m, g, 1)] = (n % 256) + 16 + (j - 1)
    bias4 = np.zeros((72, 4), np.float32)
    bias4[0:36] = np.array([0.0, 4.0, 0.0, 4.0])[None, :]
    cl = np.zeros((72, 2), np.float32)
    cl[0:36, 0] = 0.0
    cl[0:36, 1] = WIN - 2.0        # y clamp [0, 31]
    cl[36:72, 0] = 16.0
    cl[36:72, 1] = 270.0           # x clamp
    R1 = conv_r_w[:, 0:64]
    R2 = conv_r_w[:, 64:64 + Co]
    wdw = conv_m_w.reshape(64, 9)
    sig = conv_m_w.sum((1, 2, 3))
    lA = np.zeros((64, 9 * 64), np.float32)
    for m in range(9):
        lA[:, m * 64:(m + 1) * 64] = (R1 * wdw[None, :, m]).T  # [c, o]
    lB = (-R1 * sig[None, :]).T
    lR = np.zeros((72, 64), np.float32)
    for m in range(9):
        for g in range(DG):
            for c in range(2):
                lR[_q_of(m, g, c)] = R2[:, _orig_ch(m, g, c)]
    to_bf = lambda a: a.astype(ml_dtypes.bfloat16)
    lA2 = np.concatenate([lA, lA], axis=0)  # duplicate for partition-64 base
    return (to_bf(wp), biasp, cbase, bias4, cl, to_bf(lA2), to_bf(lB),
            to_bf(lR))


def kernel(x, conv_o_w, conv_o_b, conv_m_w, conv_r_w):
    x = np.asarray(x, np.float32)
    conv_o_w = np.asarray(conv_o_w, np.float32)
    conv_o_b = np.asarray(conv_o_b, np.float32)
    conv_m_w = np.asarray(conv_m_w, np.float32)
    conv_r_w = np.asarray(conv_r_w, np.float32)
    if "nc" not in _CACHE:
        _CACHE["nc"] = build_program()
    nc = _CACHE["nc"]
    wp, biasp, cbase, bias4, cl, lA, lB, lR = _prep_consts(
        conv_o_w, conv_o_b, conv_m_w, conv_r_w)

    import ml_dtypes
    xsum = x.sum(axis=1)  # [B, H, W] exact fp32 mask source
    in_maps = []
    for k in range(N_CORES):
        bb, rh = k // 2, k % 2
        r0 = rh * ROWS
        xcore = np.zeros((64, XR, 256), np.float32)
        lo, hi = r0 - 16, r0 + 145
        slo, shi = max(lo, 0), min(hi, 256)
        xcore[:, slo - lo:shi - lo, :] = x[bb, :, slo:shi, :]
        msk = (xsum[bb, r0:r0 + ROWS, :] > 0).astype(ml_dtypes.bfloat16)
        in_maps.append({
            "xc": np.ascontiguousarray(xcore.reshape(64, XR * 256)),
            "wp": wp, "biasp": biasp, "cbase": cbase, "bias4": bias4,
            "cl": cl, "lA": lA, "lB": lB, "lR": lR,
            "mskd": np.ascontiguousarray(msk.reshape(1, ROWS * 256)),
        })
    res = run_bass_kernel_spmd(nc, in_maps, list(range(N_CORES)))
    out = np.zeros((B, 64, H, W), np.float32)
    for k in range(N_CORES):
        bb, rh = k // 2, k % 2
        out[bb, :, rh * ROWS:(rh + 1) * ROWS, :] = (
            res.results[k]["outc"].reshape(64, ROWS, 256))
    return out



# revision 27
# speedup vs baseline: 1.0415x; 1.0014x over previous
import sys
sys.path.insert(0, "/opt/trn_rl_repo")
import numpy as np
from contextlib import ExitStack

import concourse.bass as bass
import concourse.bacc as bacc
import concourse.tile as tile
from concourse import mybir
from concourse.bass_utils import run_bass_kernel_spmd

# Problem constants (hardcoded per spec)
B, C, H, W = 4, 64, 256, 256
K, PAD, DG = 3, 1, 4
Co = DG * 2 * K * K  # 72
N_CORES = 8
ROWS = 128            # output rows per core
XR = 161              # xcore rows: [r0-16, r0+145)
M_OFF = 12            # gather window margin
WIN = 33              # window rows per half
NE = WIN * 256        # ap_gather num_elems (d-blocks)
NB = 8                # blocks per core
R = 16                # rows per block
NPOS = R * 256        # 4096 positions per block
NIDX = 1024           # samples per gather call per 16-partition group
f32, bf16, i16, i32, u32 = (mybir.dt.float32, mybir.dt.bfloat16,
                            mybir.dt.int16, mybir.dt.int32, mybir.dt.uint32)

_CACHE = {}


def _q_of(m, g, c):
    # t-slab partition for tap m, group g, coord c (0=y,1=x): q = c*36 + m*4 + g
    return c * 36 + m * 4 + g


def _orig_ch(m, g, c):
    # original offset-channel in reference layout: g*18 + m*2 + c
    return g * 18 + m * 2 + c


def build_program():
    nc = bacc.Bacc("TRN2", target_bir_lowering=False, debug=False,
                   num_devices=N_CORES)
    xc_ap = nc.dram_tensor("xc", [64, XR * 256], f32, kind="ExternalInput").ap()
    wp_ap = nc.dram_tensor("wp", [64, 9 * 128], bf16, kind="ExternalInput").ap()
    bias_ap = nc.dram_tensor("biasp", [73, 1], f32, kind="ExternalInput").ap()
    cb_ap = nc.dram_tensor("cbase", [72, 1024], f32, kind="ExternalInput").ap()
    b4_ap = nc.dram_tensor("bias4", [72, 4], f32, kind="ExternalInput").ap()
    cl_ap = nc.dram_tensor("cl", [72, 2], f32, kind="ExternalInput").ap()
    msk_ap = nc.dram_tensor("mskd", [1, ROWS * 256], bf16,
                            kind="ExternalInput").ap()
    la_ap = nc.dram_tensor("lA", [128, 9 * 64], bf16, kind="ExternalInput").ap()
    lb_ap = nc.dram_tensor("lB", [64, 64], bf16, kind="ExternalInput").ap()
    lr_ap = nc.dram_tensor("lR", [72, 64], bf16, kind="ExternalInput").ap()
    out_ap = nc.dram_tensor("outc", [64, ROWS * 256], f32, kind="ExternalOutput").ap()

    imgq_d = nc.dram_tensor("imgq_d", [64, 160 * 512], u32).ap()

    with ExitStack() as ctx:
        tc = ctx.enter_context(tile.TileContext(nc))
        cpool = ctx.enter_context(tc.tile_pool(name="consts", bufs=1))
        wp_t = cpool.tile([64, 9 * 128], bf16, tag="wp")
        nc.sync.dma_start(wp_t[:], wp_ap[:])
        bias_t = cpool.tile([73, 1], f32, tag="bias")
        nc.sync.dma_start(bias_t[:], bias_ap[:])
        cb_t = cpool.tile([72, 1024], f32, tag="cb")
        nc.sync.dma_start(cb_t[:], cb_ap[:])
        b4_t = cpool.tile([72, 4], f32, tag="b4")
        nc.sync.dma_start(b4_t[:], b4_ap[:])
        cl_t = cpool.tile([72, 2], f32, tag="cl")
        nc.sync.dma_start(cl_t[:], cl_ap[:])
        la_t = cpool.tile([128, 9 * 64], bf16, tag="la")
        nc.sync.dma_start(la_t[:], la_ap[:])
        lb_t = cpool.tile([64, 64], bf16, tag="lb")
        nc.sync.dma_start(lb_t[:], lb_ap[:])
        lr_t = cpool.tile([72, 64], bf16, tag="lr")
        nc.sync.dma_start(lr_t[:], lr_ap[:])
        cm1_t = cpool.tile([72, 1], f32, tag="cm1")
        nc.vector.memset(cm1_t[:], -1.0)

        # ---- prepass: build imgq_d (bf16 pair-quad image) ----
        with tc.tile_pool(name="prep", bufs=2) as ppool:
            for cch in range(8):
                r0 = 20 * cch
                xp = ppool.tile([64, 21 * 257], f32, tag="xp")
                xpv = xp[:].rearrange("p (r w) -> p r w", w=257)
                nc.sync.dma_start(
                    xpv[:, :, 0:256],
                    xc_ap[:, r0 * 256:(r0 + 21) * 256].rearrange(
                        "p (r w) -> p r w", w=256))
                nc.vector.memset(xpv[:, :, 256:257], 0.0)
                qt = ppool.tile([64, 20 * 512], u32, tag="qt")
                qv = qt[:].bitcast(bf16).rearrange(
                    "p (r w k e) -> p r w k e", w=256, k=2, e=2)
                nc.vector.tensor_copy(qv[:, :, :, 0, 0], xpv[:, 0:20, 0:256])
                nc.scalar.copy(qv[:, :, :, 0, 1], xpv[:, 0:20, 1:257])
                nc.vector.tensor_copy(qv[:, :, :, 1, 0], xpv[:, 1:21, 0:256])
                nc.scalar.copy(qv[:, :, :, 1, 1], xpv[:, 1:21, 1:257])
                nc.scalar.dma_start(imgq_d[:, r0 * 512:(r0 + 20) * 512],
                                    qt[:])

        # ---- block pools ----
        qwpool = ctx.enter_context(tc.tile_pool(name="qw", bufs=1))
        xwpool = ctx.enter_context(tc.tile_pool(name="xw", bufs=1))
        tpool = ctx.enter_context(tc.tile_pool(name="tt", bufs=1))
        psA = ctx.enter_context(tc.tile_pool(name="psA", bufs=1, space="PSUM"))
        psB = ctx.enter_context(tc.tile_pool(name="psB", bufs=1, space="PSUM"))
        psF = ctx.enter_context(tc.tile_pool(name="psF", bufs=2, space="PSUM"))
        ch_pool = ctx.enter_context(tc.tile_pool(name="chain", bufs=1))
        al_pool = ctx.enter_context(tc.tile_pool(name="aligned", bufs=1))
        ix_pool = ctx.enter_context(tc.tile_pool(name="ixp", bufs=2))
        w4pool = ctx.enter_context(tc.tile_pool(name="w4p", bufs=1))
        gpool = ctx.enter_context(tc.tile_pool(name="gp", bufs=2))
        bpool = ctx.enter_context(tc.tile_pool(name="bp", bufs=2))
        spool = ctx.enter_context(tc.tile_pool(name="sp", bufs=2))
        opool = ctx.enter_context(tc.tile_pool(name="op", bufs=2))
        mpool = ctx.enter_context(tc.tile_pool(name="mp", bufs=1))
        itpool = ctx.enter_context(tc.tile_pool(name="it", bufs=2))
        drpool = ctx.enter_context(tc.tile_pool(name="dr", bufs=2, space="DRAM"))

        for b in range(NB):
            bs = 16 * b

            # t conv + chain, per 1024-pos chunk (4 rows)
            idxcm = ix_pool.tile([36, NPOS], i16, tag="idxcm")
            w4d_b = drpool.tile([36, NPOS * 4], bf16, tag="w4d")
            tmask = mpool.tile([72, NPOS], bf16, tag="tmask")
            xblk_f = mpool.tile([64, NPOS], bf16, tag="xblkf")
            for cki, ck in enumerate((0, 2, 1, 3)):
                xw = xwpool.tile([64, 6 * 258], f32, tag="xw")
                xwv = xw[:].rearrange("p (r w) -> p r w", w=258)
                nc.sync.dma_start(
                    xwv[:, :, 1:257],
                    xc_ap[:, (bs + 15 + 4 * ck) * 256:
                          (bs + 21 + 4 * ck) * 256].rearrange(
                        "p (r w) -> p r w", w=256))
                nc.vector.memset(xwv[:, :, 0:1], 0.0)
                nc.vector.memset(xwv[:, :, 257:258], 0.0)
                nc.scalar.copy(
                    xblk_f[:, ck * 1024:(ck + 1) * 1024],
                    xwv[:, 1:5, 1:257])
                xwb = xwpool.tile([64, 6 * 258], bf16, tag="xwb")
                nc.vector.tensor_copy(xwb[:], xw[:])
                xwbv = xwb[:].rearrange("p (r w) -> p r w", w=258)
                ps = (psA if cki % 2 == 0 else psB).tile([73, 1024], f32,
                                                         tag="pst")
                for m in range(9):
                    dy, dx = m // 3 - 1, m % 3 - 1
                    for sub in range(2):
                        # rows of this sub-matmul: 2 rows = 512 cols
                        rr = 1 + dy + 2 * sub
                        rhs = xwbv[:, rr:rr + 2, 1 + dx:1 + dx + 256]
                        nc.tensor.matmul(ps[:, sub * 512:(sub + 1) * 512],
                                         wp_t[:, m * 128:m * 128 + 73],
                                         rhs, start=(m == 0), stop=(m == 8))
                t_c = tpool.tile([73, 1024], f32, tag="tc")
                nc.scalar.activation(t_c[:], ps[:],
                                     mybir.ActivationFunctionType.Identity,
                                     bias=bias_t[:])

                # host-precomputed mask plane -> bcast -> tmask chunk
                mkb = mpool.tile([72, 1024], bf16, tag="mkb")
                nc.scalar.dma_start(
                    mkb[:],
                    msk_ap[0:1, b * NPOS + ck * 1024:
                           b * NPOS + (ck + 1) * 1024].broadcast_to(
                        (72, 1024)))
                nc.vector.tensor_tensor(
                    tmask[:, ck * 1024:(ck + 1) * 1024], t_c[0:72, :],
                    mkb[:], op=mybir.AluOpType.mult)

                # ---- chain ----
                P = ch_pool.tile([72, 1024], f32, tag="P")
                nc.vector.tensor_tensor(P[:], t_c[0:72, :], cb_t[:],
                                        op=mybir.AluOpType.add)
                if ck > 0:
                    nc.vector.tensor_scalar(P[:], P[:], b4_t[:, ck:ck + 1],
                                            None, op0=mybir.AluOpType.add)
                Q = ch_pool.tile([72, 1024], f32, tag="B")
                nc.scalar.activation(Q[:], P[:],
                                     mybir.ActivationFunctionType.Copy,
                                     bias=-0.5, scale=1.0)
                I = ch_pool.tile([72, 1024], i32, tag="I")
                nc.vector.tensor_copy(I[:], Q[:])
                Jf = ch_pool.tile([72, 1024], f32, tag="Jf")
                nc.vector.tensor_copy(Jf[:], I[:])
                nc.vector.tensor_scalar(Jf[:], Jf[:], cl_t[:, 0:1],
                                        cl_t[:, 1:2],
                                        op0=mybir.AluOpType.max,
                                        op1=mybir.AluOpType.min)
                U = ch_pool.tile([72, 1024], f32, tag="B")
                nc.vector.tensor_tensor(U[:], P[:], Jf[:],
                                        op=mybir.AluOpType.subtract)

                # align x-side rows [36:72] down to partitions 0:36
                jx = al_pool.tile([36, 1024], f32, tag="jx")
                nc.scalar.dma_start(jx[:], Jf[36:72, :])

                # idx = Jy*256 + Jx - 16  (f32 exact), then -> int16
                af = ch_pool.tile([36, 1024], f32, tag="P")
                nc.vector.tensor_scalar(af[:], Jf[0:36, :], 256.0, -16.0,
                                        op0=mybir.AluOpType.mult,
                                        op1=mybir.AluOpType.add)
                nc.vector.tensor_tensor(af[:], af[:], jx[:],
                                        op=mybir.AluOpType.add)
                dst_v = idxcm[:, ck * 1024:(ck + 1) * 1024].rearrange(
                    "p (r s) -> p r s", r=16, s=64)
                src_v = af[:].rearrange("p (s r) -> p r s", s=64, r=16)
                nc.vector.tensor_copy(dst_v, src_v)

                # wA = relu(min(1-u, 1+u)) = relu(1-|u|)
                # wB = relu(min(2-u, u))   = relu(1-|u-1|)
                A1 = ch_pool.tile([72, 1024], f32, tag="I")
                nc.scalar.activation(A1[:], U[:],
                                     mybir.ActivationFunctionType.Abs,
                                     bias=0.0, scale=1.0)
                WA = ch_pool.tile([72, 1024], f32, tag="Jf")
                nc.scalar.activation(WA[:], A1[:],
                                     mybir.ActivationFunctionType.Relu,
                                     bias=1.0, scale=-1.0)
                A2 = ch_pool.tile([72, 1024], f32, tag="F")
                nc.scalar.activation(A2[:], U[:],
                                     mybir.ActivationFunctionType.Abs,
                                     bias=cm1_t[:], scale=1.0)
                WB = ch_pool.tile([72, 1024], f32, tag="B")
                nc.scalar.activation(WB[:], A2[:],
                                     mybir.ActivationFunctionType.Relu,
                                     bias=1.0, scale=-1.0)

                wxA = al_pool.tile([36, 1024], f32, tag="wxA")
                nc.scalar.dma_start(wxA[:], WA[36:72, :])
                wxB = al_pool.tile([36, 1024], f32, tag="wxB")
                nc.scalar.dma_start(wxB[:], WB[36:72, :])

                # W4 quad (interleaved bf16): order (yA*xA, yA*xB, yB*xA, yB*xB)
                w4c = w4pool.tile([36, 1024 * 4], bf16, tag="w4c")
                w4v = w4c[:].rearrange("p (n k) -> p n k", k=4)
                nc.vector.tensor_tensor(w4v[:, :, 0], WA[0:36, :], wxA[:],
                                        op=mybir.AluOpType.mult)
                nc.vector.tensor_tensor(w4v[:, :, 1], WA[0:36, :], wxB[:],
                                        op=mybir.AluOpType.mult)
                nc.vector.tensor_tensor(w4v[:, :, 2], WB[0:36, :], wxA[:],
                                        op=mybir.AluOpType.mult)
                nc.vector.tensor_tensor(w4v[:, :, 3], WB[0:36, :], wxB[:],
                                        op=mybir.AluOpType.mult)
                nc.scalar.dma_start(
                    w4d_b[:, ck * 4096:(ck + 1) * 4096], w4c[:])


            # imgQ window: xcore rows [bs+4, bs+45), duplicated to both halves
            # (emitted after the conv chunks so the sync-queue FIFO doesn't
            # head-of-line-block the next block's xw loads behind it)
            qw = qwpool.tile([128, WIN * 512], u32, tag="qw")
            nc.sync.dma_start(qw[0:64, :],
                              imgq_d[:, (bs + 4) * 512:(bs + 37) * 512])
            nc.sync.dma_start(qw[64:128, :],
                              imgq_d[:, (bs + 12) * 512:(bs + 45) * 512])

            # gather + blend + final conv
            for q in range(2):
                psq0 = psF.tile([64, 1024], f32, tag="psf")
                psq1 = psF.tile([64, 1024], f32, tag="psf")
                psq = [psq0, psq1]
                first_mm = [True, True]
                for m in range(9):
                    it = itpool.tile([128, 64], i16, tag="it")
                    for hf in range(2):
                        off = hf * 2048 + q * 1024
                        srcv = idxcm[m * 4:(m + 1) * 4,
                                     off:off + 1024].rearrange(
                                         "g (r s) -> g r s", s=64)
                        nc.sync.dma_start(it[hf * 64:(hf + 1) * 64, :], srcv)
                    w4b = bpool.tile([128, 4096], bf16, tag="w4b")
                    for hf in range(2):
                        off = (hf * 2048 + q * 1024) * 4
                        src3 = w4d_b[m * 4:(m + 1) * 4, off:off + 4096]
                        src3 = src3[:, None, :].broadcast_to((4, 16, 4096))
                        nc.scalar.dma_start(
                            w4b[hf * 64:(hf + 1) * 64, :], src3)
                    gt = gpool.tile([128, NIDX * 2], u32, tag="gt")
                    nc.gpsimd.ap_gather(gt[:], qw[:], it[:], channels=128,
                                        num_elems=NE, d=2, num_idxs=NIDX)
                    gb = gt[:].bitcast(bf16)
                    nc.vector.tensor_tensor(gb, gb, w4b[:],
                                            op=mybir.AluOpType.mult)
                    g2 = gb.rearrange("p (n k) -> p n k", k=2)
                    t1 = gb[:, 0:NIDX * 2]
                    nc.vector.tensor_tensor(t1, g2[:, :, 0], g2[:, :, 1],
                                            op=mybir.AluOpType.add)
                    t2 = t1.rearrange("p (n k) -> p n k", k=2)
                    sm = spool.tile([128, NIDX], bf16, tag="sm")
                    nc.vector.tensor_tensor(sm[:], t2[:, :, 0], t2[:, :, 1],
                                            op=mybir.AluOpType.add)
                    for hf in range(2):
                        for qq in range(2):
                            nc.tensor.matmul(
                                psq[hf][:, qq * 512:(qq + 1) * 512],
                                la_t[hf * 64:(hf + 1) * 64,
                                     m * 64:(m + 1) * 64],
                                sm[hf * 64:(hf + 1) * 64,
                                   qq * 512:(qq + 1) * 512],
                                start=first_mm[hf], stop=False)
                        first_mm[hf] = False
                # B and R2 terms for this q-region, then tanh + out
                for hf in range(2):
                    for qq in range(2):
                        cols = slice(hf * 2048 + q * 1024 + qq * 512,
                                     hf * 2048 + q * 1024 + (qq + 1) * 512)
                        pcols = slice(qq * 512, (qq + 1) * 512)
                        nc.tensor.matmul(psq[hf][:, pcols], lb_t[:],
                                         xblk_f[:, cols],
                                         start=False, stop=False)
                        nc.tensor.matmul(psq[hf][:, pcols], lr_t[:],
                                         tmask[:, cols], start=False,
                                         stop=True)
                    outsb = opool.tile([64, 1024], f32, tag="outsb")
                    nc.scalar.activation(outsb[:], psq[hf][:],
                                         mybir.ActivationFunctionType.Tanh)
                    nc.scalar.dma_start(
                        out_ap[:, b * NPOS + hf * 2048 + q * 1024:
                               b * NPOS + hf * 2048 + (q + 1) * 1024],
                        outsb[:])

    nc.compile()
    return nc


def _prep_consts(conv_o_w, conv_o_b, conv_m_w, conv_r_w):
    import ml_dtypes
    # permuted conv weights: lhsT [64, 9*128]: per tap m cols [0..73)
    Wmod = conv_o_w.copy()
    Wmod[:, :, 1, 1] -= conv_o_w.sum((2, 3))
    wp = np.zeros((64, 9 * 128), np.float32)
    biasp = np.zeros((73, 1), np.float32)
    for mt in range(9):          # conv tap
        dy, dx = mt // 3, mt % 3
        for mq in range(9):      # output offset-channel tap slot
            for g in range(DG):
                for c in range(2):
                    q = _q_of(mq, g, c)
                    oc = _orig_ch(mq, g, c)
                    wp[:, mt * 128 + q] = Wmod[oc, :, dy, dx]
    for mq in range(9):
        for g in range(DG):
            for c in range(2):
                biasp[_q_of(mq, g, c), 0] = conv_o_b[_orig_ch(mq, g, c)]
    wp[:, 4 * 128 + 72] = 1.0  # xsum via center tap
    biasp[72, 0] = 0.0

    cbase = np.zeros((72, 1024), np.float32)
    n = np.arange(1024)
    for m in range(9):
        i, j = m // 3, m % 3
        for g in range(DG):
            cbase[_q_of(m, g, 0)] = (n // 256) + M_OFF + (i - 1)
            cbase[_q_of(m, g, 1)] = (n % 256) + 16 + (j - 1)
    bias4 = np.zeros((72, 4), np.float32)
    bias4[0:36] = np.array([0.0, 4.0, 0.0, 4.0])[None, :]
    cl = np.zeros((72, 2), np.float32)
    cl[0:36, 0] = 0.0
    cl[0:36, 1] = WIN - 2.0        # y clamp [0, 31]
    cl[36:72, 0] = 16.0
    cl[36:72, 1] = 270.0           # x clamp
    R1 = conv_r_w[:, 0:64]
    R2 = conv_r_w[:, 64:64 + Co]
    wdw = conv_m_w.reshape(64, 9)
    sig = conv_m_w.sum((1, 2, 3))
    lA = np.zeros((64, 9 * 64), np.float32)
    for m in range(9):
        lA[:, m * 64:(m + 1) * 64] = (R1 * wdw[None, :, m]).T  # [c, o]
    lB = (-R1 * sig[None, :]).T
    lR = np.zeros((72, 64), np.float32)
    for m in range(9):
        for g in range(DG):
            for c in range(2):
                lR[_q_of(m, g, c)] = R2[:, _orig_ch(m, g, c)]
    to_bf = lambda a: a.astype(ml_dtypes.bfloat16)
    lA2 = np.concatenate([lA, lA], axis=0)  # duplicate for partition-64 base
    return (to_bf(wp), biasp, cbase, bias4, cl, to_bf(lA2), to_bf(lB),
            to_bf(lR))


def kernel(x, conv_o_w, conv_o_b, conv_m_w, conv_r_w):
    x = np.asarray(x, np.float32)
    conv_o_w = np.asarray(conv_o_w, np.float32)
    conv_o_b = np.asarray(conv_o_b, np.float32)
    conv_m_w = np.asarray(conv_m_w, np.float32)
    conv_r_w = np.asarray(conv_r_w, np.float32)
    if "nc" not in _CACHE:
        _CACHE["nc"] = build_program()
    nc = _CACHE["nc"]
    wp, biasp, cbase, bias4, cl, lA, lB, lR = _prep_consts(
        conv_o_w, conv_o_b, conv_m_w, conv_r_w)

    import ml_dtypes
    xsum = x.sum(axis=1)  # [B, H, W] exact fp32 mask source
    in_maps = []
    for k in range(N_CORES):
        bb, rh = k // 2, k % 2
        r0 = rh * ROWS
        xcore = np.zeros((64, XR, 256), np.float32)
        lo, hi = r0 - 16, r0 + 145
        slo, shi = max(lo, 0), min(hi, 256)
        xcore[:, slo - lo:shi - lo, :] = x[bb, :, slo:shi, :]
        msk = (xsum[bb, r0:r0 + ROWS, :] > 0).astype(ml_dtypes.bfloat16)
        in_maps.append({
            "xc": np.ascontiguousarray(xcore.reshape(64, XR * 256)),
            "wp": wp, "biasp": biasp, "cbase": cbase, "bias4": bias4,
            "cl": cl, "lA": lA, "lB": lB, "lR": lR,
            "mskd": np.ascontiguousarray(msk.reshape(1, ROWS * 256)),
        })
    res = run_bass_kernel_spmd(nc, in_maps, list(range(N_CORES)))
    out = np.zeros((B, 64, H, W), np.float32)
    for k in range(N_CORES):
        bb, rh = k // 2, k % 2
        out[bb, :, rh * ROWS:(rh + 1) * ROWS, :] = (
            res.results[k]["outc"].reshape(64, ROWS, 256))
    return out

